# revision 10
# baseline (speedup 1.0000x reference)
"""CRF forward-algorithm kernel for Trainium2 (8 NeuronCores, Bass).

Strategy: data-parallel over batch (32 -> 4 per core). The per-step
recursion  alpha_t[b,j] = scores[b,t,j] + lse_i(trans[i,j] + alpha_{t-1}[b,i])
is run in linear space with a global per-step normalizer K:

    p_t[j,b] = exp(scores[b,t,j] - K) * sum_i E[i,j] * p_{t-1}[i,b]
    alpha[b,t,j] = ln(p_t[j,b]) + K*t  (+ -10000 on the j==0 lane)

where E = exp(trans) with column 0 (trans == -10000 exactly) replaced by 1
and row 0 zeroed (its true contribution underflows to 0 in f32 anyway).
Per step this is one PE matmul (E stationary) + one DVE multiply; the ln,
the K*t correction, output transposes and DMAs are bulk work off the
sequential chain.
"""

import numpy as np

N = 64
T = 512
B = 32
NCORES = 8
BS = B // NCORES  # 4 batch elements per core
K = 4.66


def _build_program():
    import concourse.bass as bass
    import concourse.mybir as mybir

    FT = mybir.dt.float32
    AF = mybir.ActivationFunctionType

    nc = bass.Bass()
    sc_d = nc.declare_dram_parameter("sc", [BS * T, N], FT, isOutput=False)
    tr_d = nc.declare_dram_parameter("tr", [N, N], FT, isOutput=False)
    trt_d = nc.declare_dram_parameter("trt", [N, N], FT, isOutput=False)
    ktc_d = nc.declare_dram_parameter("ktc", [N, T], FT, isOutput=False)
    id_d = nc.declare_dram_parameter("ident", [128, 128], FT, isOutput=False)
    kc_d = nc.declare_dram_parameter("kconst", [N, 2], FT, isOutput=False)
    out_d = nc.declare_dram_parameter("out", [BS * T, N], FT, isOutput=True)

    from contextlib import ExitStack

    with ExitStack() as ctx:
        sc_nat = ctx.enter_context(nc.sbuf_tensor([128, 16 * N], FT))
        es_all = ctx.enter_context(nc.sbuf_tensor([N, BS * T], FT))
        p_all = ctx.enter_context(nc.sbuf_tensor([N, BS * T], FT))
        out_sb = ctx.enter_context(nc.sbuf_tensor([N, BS * T], FT))
        e_sb = ctx.enter_context(nc.sbuf_tensor([N, N], FT))
        tr_nat = ctx.enter_context(nc.sbuf_tensor([N, N], FT))
        tr_t = ctx.enter_context(nc.sbuf_tensor([N, N], FT))
        e0k = ctx.enter_context(nc.sbuf_tensor([N, 1], FT))
        ktc_sb = ctx.enter_context(nc.sbuf_tensor([N, T], FT))
        ident = ctx.enter_context(nc.sbuf_tensor([128, 128], FT))
        out_tr = ctx.enter_context(nc.sbuf_tensor([128, 16 * N], FT))
        kc_sb = ctx.enter_context(nc.sbuf_tensor([N, 2], FT))
        tp0 = ctx.enter_context(nc.psum_tensor([N, 128], FT))
        tp1 = ctx.enter_context(nc.psum_tensor([N, 128], FT))
        s_ps = ctx.enter_context(nc.psum_tensor([N, BS], FT))
        tq0 = ctx.enter_context(nc.psum_tensor([128, N], FT))
        tq1 = ctx.enter_context(nc.psum_tensor([128, N], FT))
        dma_sem = ctx.enter_context(nc.semaphore())
        acte_sem = ctx.enter_context(nc.semaphore())
        act_sem = ctx.enter_context(nc.semaphore())
        dve_sem = ctx.enter_context(nc.semaphore())
        pe_sem = ctx.enter_context(nc.semaphore())
        actln_sem = ctx.enter_context(nc.semaphore())
        dvek_sem = ctx.enter_context(nc.semaphore())
        pe2_sem = ctx.enter_context(nc.semaphore())
        out_sem = ctx.enter_context(nc.semaphore())
        actcp_sem = ctx.enter_context(nc.semaphore())
        block = ctx.enter_context(nc.Block())
        tp = [tp0, tp1]
        tq = [tq0, tq1]
        # [j, t, b] step views of the [j, b*T + t] flat free layout
        esv = es_all[:, :].rearrange("p (b t) -> p t b", t=T)
        pv = p_all[:, :].rearrange("p (b t) -> p t b", t=T)

        @block.sync
        def _(sync):
            sync.dma_start(
                sc_nat[:, :].rearrange("p (k j) -> p k j", j=N),
                sc_d[:, :].rearrange("(k p) j -> p k j", p=128),
            ).then_inc(dma_sem, 16)
            sync.dma_start(tr_nat[:, :], tr_d[:, :]).then_inc(dma_sem, 16)
            sync.dma_start(tr_t[:, :], trt_d[:, :]).then_inc(dma_sem, 16)
            sync.dma_start(ktc_sb[:, :], ktc_d[:, :]).then_inc(dma_sem, 16)
            sync.dma_start(ident[:, :], id_d[:, :]).then_inc(dma_sem, 16)
            sync.dma_start(kc_sb[:, :], kc_d[:, :]).then_inc(dma_sem, 16)
            out_v = out_d[:, :].rearrange("(k p) j -> k p j", p=128)
            for k in range(16):
                sync.wait_ge(actcp_sem, k + 1)
                sync.dma_start(
                    out_v[k], out_tr[:, k * N : (k + 1) * N]
                ).then_inc(out_sem, 16)

        @block.tensor
        def _(tensor):
            tensor.wait_ge(dma_sem, 96)
            # scores tiles [128(bt), 64(j)] -> psum [64(j), 128(bt)]
            for k in range(16):
                if k >= 2:
                    tensor.wait_ge(act_sem, k - 1)
                tensor.transpose(
                    tp[k % 2][:, :], sc_nat[:, k * N : (k + 1) * N], ident[:, :]
                ).then_inc(pe_sem, 1)
            # sequential scan: s = E^T @ p_{t-1}
            for t in range(1, T):
                tensor.wait_ge(dve_sem, t)
                tensor.matmul(s_ps[:, :], e_sb[:, :], pv[:, t - 1, :]).then_inc(
                    pe_sem, 1
                )
            # output transposes [64(j), 128(bt)] -> psum [128(bt), 64(j)]
            for k in range(16):
                tensor.wait_ge(dvek_sem, k // 4 + 1)
                if k >= 2:
                    tensor.wait_ge(actcp_sem, k - 1)
                tensor.transpose(
                    tq[k % 2][:, :], out_sb[:, k * 128 : (k + 1) * 128],
                    ident[0:N, 0:N],
                ).then_inc(pe2_sem, 1)

        @block.scalar
        def _(scalar):
            scalar.wait_ge(dma_sem, 96)
            scalar.activation(e_sb[:, :], tr_nat[:, :], AF.Exp).then_inc(acte_sem, 1)
            scalar.activation(
                e0k[:, :], tr_t[:, 0:1], AF.Exp, bias=kc_sb[:, 0:1]
            ).then_inc(acte_sem, 1)
            for k in range(16):
                scalar.wait_ge(pe_sem, k + 1)
                scalar.activation(
                    es_all[:, k * 128 : (k + 1) * 128], tp[k % 2][:, :], AF.Exp,
                    bias=kc_sb[:, 1:2],
                ).then_inc(act_sem, 1)
            scalar.wait_ge(dve_sem, T)
            for c in range(BS):
                scalar.activation(
                    out_sb[:, c * T : (c + 1) * T], p_all[:, c * T : (c + 1) * T],
                    AF.Ln,
                ).then_inc(actln_sem, 1)
            for k in range(16):
                scalar.wait_ge(pe2_sem, k + 1)
                scalar.copy(
                    out_tr[:, k * N : (k + 1) * N], tq[k % 2][:, :]
                ).then_inc(actcp_sem, 1)

        @block.vector
        def _(vector):
            vector.wait_ge(acte_sem, 2)
            vector.wait_ge(act_sem, 16)
            vector.memset(e_sb[:, 0:1], 1.0)
            vector.memset(e_sb[0:1, :], 0.0)
            vector.memset(e0k[0:1, 0:1], float(np.exp(K)))
            vector.tensor_scalar_mul(pv[:, 0, :], esv[:, 0, :], e0k[:, :]).then_inc(
                dve_sem, 1
            )
            for t in range(1, T):
                vector.wait_ge(pe_sem, 16 + t)
                vector.tensor_mul(pv[:, t, :], s_ps[:, :], esv[:, t, :]).then_inc(
                    dve_sem, 1
                )
            for c in range(BS):
                vector.wait_ge(actln_sem, c + 1)
                vector.tensor_add(
                    out_sb[:, c * T : (c + 1) * T],
                    out_sb[:, c * T : (c + 1) * T],
                    ktc_sb[:, :],
                ).then_inc(dvek_sem, 1)

    return nc


LAST_RESULT = None


def kernel(scores: np.ndarray, transitions: np.ndarray) -> np.ndarray:
    global LAST_RESULT
    from concourse.bass_utils import run_bass_kernel_spmd

    scores = np.ascontiguousarray(scores, dtype=np.float32)
    transitions = np.ascontiguousarray(transitions, dtype=np.float32)

    ktc = (K * np.arange(T, dtype=np.float32))[None, :] * np.ones(
        (N, 1), dtype=np.float32
    )
    ktc[0, :] -= 10000.0
    ident = np.eye(128, dtype=np.float32)
    kconst = np.stack([np.full(N, K, np.float32), np.full(N, -K, np.float32)], axis=1)
    trt = np.ascontiguousarray(transitions.T)

    nc = _build_program()
    in_maps = []
    for c in range(NCORES):
        shard = np.ascontiguousarray(
            scores[c * BS : (c + 1) * BS].reshape(BS * T, N)
        )
        in_maps.append(
            {"sc": shard, "tr": transitions, "trt": trt, "ktc": ktc,
             "ident": ident, "kconst": kconst}
        )
    res = run_bass_kernel_spmd(nc, in_maps, list(range(NCORES)))
    LAST_RESULT = res
    out = np.empty((B, T, N), dtype=np.float32)
    for c in range(NCORES):
        out[c * BS : (c + 1) * BS] = res.results[c]["out"].reshape(BS, T, N)
    return out


# revision 11
# speedup vs baseline: 1.2321x; 1.2321x over previous
"""CRF forward-algorithm kernel for Trainium2 (8 NeuronCores, Bass).

Strategy: data-parallel over batch (32 -> 4 per core). The per-step
recursion  alpha_t[b,j] = scores[b,t,j] + lse_i(trans[i,j] + alpha_{t-1}[b,i])
is run in linear space with a global per-step normalizer K:

    p_t[j,b] = exp(scores[b,t,j] - K) * sum_i E[i,j] * p_{t-1}[i,b]
    alpha[b,t,j] = ln(p_t[j,b]) + K*t  (+ -10000 on the j==0 lane)

where E = exp(trans) with column 0 (trans == -10000 exactly) replaced by 1
and row 0 zeroed (its true contribution underflows to 0 in f32 anyway).
E and the p state are kept in bf16 so the per-step matmul is a single
1-pass PE instruction (fp32 moving operands cost 2 half-speed passes and
double-width weight loads); the log-domain outputs only see the bf16
quantization as ~2^-9 relative noise on p, a ~1e-5 absolute error.
Per step this is one PE matmul (E stationary) + one DVE multiply; the ln,
the K*t correction, output transposes and DMAs are bulk work off the
sequential chain.
"""

import numpy as np

N = 64
T = 512
B = 32
NCORES = 8
BS = B // NCORES  # 4 batch elements per core
K = 4.66


def _build_program():
    import concourse.bass as bass
    import concourse.mybir as mybir

    FT = mybir.dt.float32
    BF = mybir.dt.bfloat16
    AF = mybir.ActivationFunctionType

    nc = bass.Bass()
    sc_d = nc.declare_dram_parameter("sc", [BS * T, N], FT, isOutput=False)
    tr_d = nc.declare_dram_parameter("tr", [N, N], FT, isOutput=False)
    trt_d = nc.declare_dram_parameter("trt", [N, N], FT, isOutput=False)
    ktc_d = nc.declare_dram_parameter("ktc", [N, T], FT, isOutput=False)
    id_d = nc.declare_dram_parameter("ident", [128, 128], FT, isOutput=False)
    kc_d = nc.declare_dram_parameter("kconst", [N, 2], FT, isOutput=False)
    out_d = nc.declare_dram_parameter("out", [BS * T, N], FT, isOutput=True)

    from contextlib import ExitStack

    with ExitStack() as ctx:
        sc_nat = ctx.enter_context(nc.sbuf_tensor([128, 16 * N], FT))
        es_all = ctx.enter_context(nc.sbuf_tensor([N, T * BS], FT))
        p_all = ctx.enter_context(nc.sbuf_tensor([N, T * BS], BF))
        out_sb = ctx.enter_context(nc.sbuf_tensor([N, T * BS], FT))
        e_sb = ctx.enter_context(nc.sbuf_tensor([N, N], BF))
        tr_nat = ctx.enter_context(nc.sbuf_tensor([N, N], FT))
        tr_t = ctx.enter_context(nc.sbuf_tensor([N, N], FT))
        e0k = ctx.enter_context(nc.sbuf_tensor([N, 1], FT))
        ktc_sb = ctx.enter_context(nc.sbuf_tensor([N, T], FT))
        ident = ctx.enter_context(nc.sbuf_tensor([128, 128], FT))
        out_tr = ctx.enter_context(nc.sbuf_tensor([128, 16 * N], FT))
        kc_sb = ctx.enter_context(nc.sbuf_tensor([N, 2], FT))
        tp0 = ctx.enter_context(nc.psum_tensor([N, 128], FT))
        tp1 = ctx.enter_context(nc.psum_tensor([N, 128], FT))
        s_ps = ctx.enter_context(nc.psum_tensor([N, BS], FT))
        tq0 = ctx.enter_context(nc.psum_tensor([128, N], FT))
        tq1 = ctx.enter_context(nc.psum_tensor([128, N], FT))
        dma_sem = ctx.enter_context(nc.semaphore())
        acte_sem = ctx.enter_context(nc.semaphore())
        act_sem = ctx.enter_context(nc.semaphore())
        dve_sem = ctx.enter_context(nc.semaphore())
        pe_sem = ctx.enter_context(nc.semaphore())
        actln_sem = ctx.enter_context(nc.semaphore())
        dvek_sem = ctx.enter_context(nc.semaphore())
        pe2_sem = ctx.enter_context(nc.semaphore())
        out_sem = ctx.enter_context(nc.semaphore())
        actcp_sem = ctx.enter_context(nc.semaphore())
        block = ctx.enter_context(nc.Block())
        tp = [tp0, tp1]
        tq = [tq0, tq1]
        # t-major free layout [j, t*BS + b]: per-step slices are contiguous,
        # per-(b, t-chunk) views are stride-BS
        esw = es_all[:, :].rearrange("p (t b) -> p b t", b=BS)
        ow = out_sb[:, :].rearrange("p (t b) -> p b t", b=BS)

        @block.sync
        def _(sync):
            sync.dma_start(
                sc_nat[:, :].rearrange("p (k j) -> p k j", j=N),
                sc_d[:, :].rearrange("(k p) j -> p k j", p=128),
            ).then_inc(dma_sem, 16)
            sync.dma_start(tr_nat[:, :], tr_d[:, :]).then_inc(dma_sem, 16)
            sync.dma_start(tr_t[:, :], trt_d[:, :]).then_inc(dma_sem, 16)
            sync.dma_start(ktc_sb[:, :], ktc_d[:, :]).then_inc(dma_sem, 16)
            sync.dma_start(ident[:, :], id_d[:, :]).then_inc(dma_sem, 16)
            sync.dma_start(kc_sb[:, :], kc_d[:, :]).then_inc(dma_sem, 16)
            out_v = out_d[:, :].rearrange("(k p) j -> k p j", p=128)
            for k in range(16):
                sync.wait_ge(actcp_sem, k + 1)
                sync.dma_start(
                    out_v[k], out_tr[:, k * N : (k + 1) * N]
                ).then_inc(out_sem, 16)

        @block.tensor
        def _(tensor):
            tensor.wait_ge(dma_sem, 96)
            # scores tiles [128(bt), 64(j)] -> psum [64(j), 128(t-sub)]
            for k in range(16):
                if k >= 2:
                    tensor.wait_ge(act_sem, k - 1)
                tensor.transpose(
                    tp[k % 2][:, :], sc_nat[:, k * N : (k + 1) * N], ident[:, :]
                ).then_inc(pe_sem, 1)
            # sequential scan: s = E^T @ p_{t-1}, E stationary bf16, 1 pass
            for t in range(1, T):
                tensor.wait_ge(dve_sem, t)
                tensor.matmul(
                    s_ps[:, :], e_sb[:, :], p_all[:, (t - 1) * BS : t * BS]
                ).then_inc(pe_sem, 1)
            # output transposes [64(j), 128(t-sub)] -> psum [128(t-sub), 64(j)]
            for k in range(16):
                b, tc = k // 4, k % 4
                tensor.wait_ge(dvek_sem, b + 1)
                if k >= 2:
                    tensor.wait_ge(actcp_sem, k - 1)
                tensor.transpose(
                    tq[k % 2][:, :], ow[:, b, tc * 128 : (tc + 1) * 128],
                    ident[0:N, 0:N],
                ).then_inc(pe2_sem, 1)

        @block.scalar
        def _(scalar):
            scalar.wait_ge(dma_sem, 96)
            scalar.activation(e_sb[:, :], tr_nat[:, :], AF.Exp).then_inc(acte_sem, 1)
            scalar.activation(
                e0k[:, :], tr_t[:, 0:1], AF.Exp, bias=kc_sb[:, 0:1]
            ).then_inc(acte_sem, 1)
            for k in range(16):
                b, tc = k // 4, k % 4
                scalar.wait_ge(pe_sem, k + 1)
                scalar.activation(
                    esw[:, b, tc * 128 : (tc + 1) * 128], tp[k % 2][:, :], AF.Exp,
                    bias=kc_sb[:, 1:2],
                ).then_inc(act_sem, 1)
            scalar.wait_ge(dve_sem, T)
            scalar.activation(out_sb[:, :], p_all[:, :], AF.Ln).then_inc(
                actln_sem, 1
            )
            for k in range(16):
                scalar.wait_ge(pe2_sem, k + 1)
                scalar.copy(
                    out_tr[:, k * N : (k + 1) * N], tq[k % 2][:, :]
                ).then_inc(actcp_sem, 1)

        @block.vector
        def _(vector):
            vector.wait_ge(acte_sem, 2)
            vector.wait_ge(act_sem, 16)
            vector.memset(e_sb[:, 0:1], 1.0)
            vector.memset(e_sb[0:1, :], 0.0)
            vector.memset(e0k[0:1, 0:1], float(np.exp(K)))
            vector.tensor_scalar_mul(
                p_all[:, 0:BS], es_all[:, 0:BS], e0k[:, :]
            ).then_inc(dve_sem, 1)
            for t in range(1, T):
                vector.wait_ge(pe_sem, 16 + t)
                vector.tensor_mul(
                    p_all[:, t * BS : (t + 1) * BS],
                    s_ps[:, :],
                    es_all[:, t * BS : (t + 1) * BS],
                ).then_inc(dve_sem, 1)
            for c in range(BS):
                vector.wait_ge(actln_sem, 1)
                vector.tensor_add(
                    ow[:, c, :], ow[:, c, :], ktc_sb[:, :]
                ).then_inc(dvek_sem, 1)

    return nc


LAST_RESULT = None


def kernel(scores: np.ndarray, transitions: np.ndarray) -> np.ndarray:
    global LAST_RESULT
    from concourse.bass_utils import run_bass_kernel_spmd

    scores = np.ascontiguousarray(scores, dtype=np.float32)
    transitions = np.ascontiguousarray(transitions, dtype=np.float32)

    ktc = (K * np.arange(T, dtype=np.float32))[None, :] * np.ones(
        (N, 1), dtype=np.float32
    )
    ktc[0, :] -= 10000.0
    ident = np.eye(128, dtype=np.float32)
    kconst = np.stack([np.full(N, K, np.float32), np.full(N, -K, np.float32)], axis=1)
    trt = np.ascontiguousarray(transitions.T)

    nc = _build_program()
    in_maps = []
    for c in range(NCORES):
        shard = np.ascontiguousarray(
            scores[c * BS : (c + 1) * BS].reshape(BS * T, N)
        )
        in_maps.append(
            {"sc": shard, "tr": transitions, "trt": trt, "ktc": ktc,
             "ident": ident, "kconst": kconst}
        )
    res = run_bass_kernel_spmd(nc, in_maps, list(range(NCORES)))
    LAST_RESULT = res
    out = np.empty((B, T, N), dtype=np.float32)
    for c in range(NCORES):
        out[c * BS : (c + 1) * BS] = res.results[c]["out"].reshape(BS, T, N)
    return out


# revision 13
# speedup vs baseline: 1.5947x; 1.2942x over previous
"""CRF forward-algorithm kernel for Trainium2 (8 NeuronCores, Bass).

Strategy: data-parallel over batch (32 -> 4 per core). The per-step
recursion  alpha_t[b,j] = scores[b,t,j] + lse_i(trans[i,j] + alpha_{t-1}[b,i])
is run in linear space with a global per-step normalizer K:

    p_t[j,b] = exp(scores[b,t,j] - K) * sum_i E[i,j] * p_{t-1}[i,b]
    alpha[b,t,j] = ln(p_t[j,b]) + K*t  (+ -10000 on the j==0 lane)

where E = exp(trans) with column 0 (trans == -10000 exactly) replaced by 1
and row 0 zeroed (its true contribution underflows to 0 in f32 anyway).
E and the p state are kept in bf16 so the per-step matmul is a single
1-pass PE instruction (fp32 moving operands cost 2 half-speed passes and
double-width weight loads); the log-domain outputs only see the bf16
quantization as ~2^-9 relative noise on p, a ~1e-5 absolute error.
Per step this is one PE matmul (E stationary) + one DVE multiply; the ln,
the K*t correction, output transposes and DMAs are bulk work off the
sequential chain.
"""

import numpy as np

N = 64
T = 512
B = 32
NCORES = 8
BS = B // NCORES  # 4 batch elements per core
K = 4.66


def _build_program():
    import concourse.bass as bass
    import concourse.mybir as mybir

    FT = mybir.dt.float32
    BF = mybir.dt.bfloat16
    AF = mybir.ActivationFunctionType

    nc = bass.Bass()
    sc_d = nc.declare_dram_parameter("sc", [BS * T, N], FT, isOutput=False)
    tr_d = nc.declare_dram_parameter("tr", [N, N], FT, isOutput=False)
    trt_d = nc.declare_dram_parameter("trt", [N, N], FT, isOutput=False)
    ktc_d = nc.declare_dram_parameter("ktc", [N, T], FT, isOutput=False)
    id_d = nc.declare_dram_parameter("ident", [128, 128], FT, isOutput=False)
    kc_d = nc.declare_dram_parameter("kconst", [N, 2], FT, isOutput=False)
    out_d = nc.declare_dram_parameter("out", [BS * T, N], FT, isOutput=True)

    from contextlib import ExitStack

    with ExitStack() as ctx:
        sc_nat = ctx.enter_context(nc.sbuf_tensor([128, 16 * N], FT))
        es_all = ctx.enter_context(nc.sbuf_tensor([N, T * BS], FT))
        p_all = ctx.enter_context(nc.sbuf_tensor([N, T * BS], BF))
        out_sb = ctx.enter_context(nc.sbuf_tensor([N, T * BS], FT))
        e_sb = ctx.enter_context(nc.sbuf_tensor([N, N], BF))
        tr_nat = ctx.enter_context(nc.sbuf_tensor([N, N], FT))
        tr_t = ctx.enter_context(nc.sbuf_tensor([N, N], FT))
        e0k = ctx.enter_context(nc.sbuf_tensor([N, 1], FT))
        ktc_sb = ctx.enter_context(nc.sbuf_tensor([N, T], FT))
        ident = ctx.enter_context(nc.sbuf_tensor([128, 128], FT))
        out_tr = ctx.enter_context(nc.sbuf_tensor([128, 16 * N], FT))
        kc_sb = ctx.enter_context(nc.sbuf_tensor([N, 2], FT))
        tp0 = ctx.enter_context(nc.psum_tensor([N, 128], FT))
        tp1 = ctx.enter_context(nc.psum_tensor([N, 128], FT))
        s_ps = ctx.enter_context(nc.psum_tensor([N, BS], FT))
        tq0 = ctx.enter_context(nc.psum_tensor([128, N], FT))
        tq1 = ctx.enter_context(nc.psum_tensor([128, N], FT))
        dma_sem = ctx.enter_context(nc.semaphore())
        acte_sem = ctx.enter_context(nc.semaphore())
        act_sem = ctx.enter_context(nc.semaphore())
        dve_sem = ctx.enter_context(nc.semaphore())
        pe_sem = ctx.enter_context(nc.semaphore())
        actln_sem = ctx.enter_context(nc.semaphore())
        dvek_sem = ctx.enter_context(nc.semaphore())
        pe2_sem = ctx.enter_context(nc.semaphore())
        out_sem = ctx.enter_context(nc.semaphore())
        actcp_sem = ctx.enter_context(nc.semaphore())
        block = ctx.enter_context(nc.Block())
        tp = [tp0, tp1]
        tq = [tq0, tq1]
        # t-major free layout [j, t*BS + b]: per-step slices are contiguous,
        # per-(b, t-chunk) views are stride-BS
        esw = es_all[:, :].rearrange("p (t b) -> p b t", b=BS)
        ow = out_sb[:, :].rearrange("p (t b) -> p b t", b=BS)

        @block.sync
        def _(sync):
            sync.dma_start(
                sc_nat[:, :].rearrange("p (k j) -> p k j", j=N),
                sc_d[:, :].rearrange("(k p) j -> p k j", p=128),
            ).then_inc(dma_sem, 16)
            sync.dma_start(tr_nat[:, :], tr_d[:, :]).then_inc(dma_sem, 16)
            sync.dma_start(tr_t[:, :], trt_d[:, :]).then_inc(dma_sem, 16)
            sync.dma_start(ktc_sb[:, :], ktc_d[:, :]).then_inc(dma_sem, 16)
            sync.dma_start(ident[:, :], id_d[:, :]).then_inc(dma_sem, 16)
            sync.dma_start(kc_sb[:, :], kc_d[:, :]).then_inc(dma_sem, 16)
            out_v = out_d[:, :].rearrange("(k p) j -> k p j", p=128)
            for k in range(16):
                sync.wait_ge(actcp_sem, k + 1)
                sync.dma_start(
                    out_v[k], out_tr[:, k * N : (k + 1) * N]
                ).then_inc(out_sem, 16)

        @block.tensor
        def _(tensor):
            tensor.wait_ge(dma_sem, 96)
            # scores tiles [128(bt), 64(j)] -> psum [64(j), 128(t-sub)]
            for k in range(16):
                if k >= 2:
                    tensor.wait_ge(act_sem, k - 1)
                tensor.transpose(
                    tp[k % 2][:, :], sc_nat[:, k * N : (k + 1) * N], ident[:, :]
                ).then_inc(pe_sem, 1)
            # sequential scan: s = E^T @ p_{t-1}, E stationary bf16, 1 pass.
            # The wait is attached to the matmul itself (no standalone
            # EventSemaphore instruction on the chain).
            for t in range(1, T):
                mm = tensor.matmul(
                    s_ps[:, :], e_sb[:, :], p_all[:, (t - 1) * BS : t * BS]
                )
                mm._wait_ge(dve_sem, t)
                mm.then_inc(pe_sem, 1)
            # output transposes [64(j), 128(t-sub)] -> psum [128(t-sub), 64(j)]
            for k in range(16):
                b, tc = k // 4, k % 4
                tensor.wait_ge(dvek_sem, b + 1)
                if k >= 2:
                    tensor.wait_ge(actcp_sem, k - 1)
                tensor.transpose(
                    tq[k % 2][:, :], ow[:, b, tc * 128 : (tc + 1) * 128],
                    ident[0:N, 0:N],
                ).then_inc(pe2_sem, 1)

        @block.scalar
        def _(scalar):
            scalar.wait_ge(dma_sem, 96)
            scalar.activation(e_sb[:, :], tr_nat[:, :], AF.Exp).then_inc(acte_sem, 1)
            scalar.activation(
                e0k[:, :], tr_t[:, 0:1], AF.Exp, bias=kc_sb[:, 0:1]
            ).then_inc(acte_sem, 1)
            for k in range(16):
                b, tc = k // 4, k % 4
                scalar.wait_ge(pe_sem, k + 1)
                scalar.activation(
                    esw[:, b, tc * 128 : (tc + 1) * 128], tp[k % 2][:, :], AF.Exp,
                    bias=kc_sb[:, 1:2],
                ).then_inc(act_sem, 1)
            scalar.wait_ge(dve_sem, T)
            scalar.activation(out_sb[:, :], p_all[:, :], AF.Ln).then_inc(
                actln_sem, 1
            )
            for k in range(16):
                scalar.wait_ge(pe2_sem, k + 1)
                scalar.copy(
                    out_tr[:, k * N : (k + 1) * N], tq[k % 2][:, :]
                ).then_inc(actcp_sem, 1)

        @block.vector
        def _(vector):
            vector.wait_ge(acte_sem, 2)
            vector.wait_ge(act_sem, 16)
            vector.memset(e_sb[:, 0:1], 1.0)
            vector.memset(e_sb[0:1, :], 0.0)
            vector.memset(e0k[0:1, 0:1], float(np.exp(K)))
            vector.tensor_scalar_mul(
                p_all[:, 0:BS], es_all[:, 0:BS], e0k[:, :]
            ).then_inc(dve_sem, 1)
            for t in range(1, T):
                mul = vector.tensor_mul(
                    p_all[:, t * BS : (t + 1) * BS],
                    s_ps[:, :],
                    es_all[:, t * BS : (t + 1) * BS],
                )
                mul._wait_ge(pe_sem, 16 + t)
                mul.then_inc(dve_sem, 1)
            for c in range(BS):
                vector.wait_ge(actln_sem, 1)
                vector.tensor_add(
                    ow[:, c, :], ow[:, c, :], ktc_sb[:, :]
                ).then_inc(dvek_sem, 1)

    return nc


LAST_RESULT = None


def kernel(scores: np.ndarray, transitions: np.ndarray) -> np.ndarray:
    global LAST_RESULT
    from concourse.bass_utils import run_bass_kernel_spmd

    scores = np.ascontiguousarray(scores, dtype=np.float32)
    transitions = np.ascontiguousarray(transitions, dtype=np.float32)

    ktc = (K * np.arange(T, dtype=np.float32))[None, :] * np.ones(
        (N, 1), dtype=np.float32
    )
    ktc[0, :] -= 10000.0
    ident = np.eye(128, dtype=np.float32)
    kconst = np.stack([np.full(N, K, np.float32), np.full(N, -K, np.float32)], axis=1)
    trt = np.ascontiguousarray(transitions.T)

    nc = _build_program()
    in_maps = []
    for c in range(NCORES):
        shard = np.ascontiguousarray(
            scores[c * BS : (c + 1) * BS].reshape(BS * T, N)
        )
        in_maps.append(
            {"sc": shard, "tr": transitions, "trt": trt, "ktc": ktc,
             "ident": ident, "kconst": kconst}
        )
    res = run_bass_kernel_spmd(nc, in_maps, list(range(NCORES)))
    LAST_RESULT = res
    out = np.empty((B, T, N), dtype=np.float32)
    for c in range(NCORES):
        out[c * BS : (c + 1) * BS] = res.results[c]["out"].reshape(BS, T, N)
    return out


# revision 15
# speedup vs baseline: 1.6012x; 1.0041x over previous
"""CRF forward-algorithm kernel for Trainium2 (8 NeuronCores, Bass).

Strategy: data-parallel over batch (32 -> 4 per core). The per-step
recursion  alpha_t[b,j] = scores[b,t,j] + lse_i(trans[i,j] + alpha_{t-1}[b,i])
is run in linear space with a global per-step normalizer K:

    p_t[j,b] = exp(scores[b,t,j] - K) * sum_i E[i,j] * p_{t-1}[i,b]
    alpha[b,t,j] = ln(p_t[j,b]) + K*t  (+ -10000 on the j==0 lane)

where E = exp(trans) with column 0 (trans == -10000 exactly) replaced by 1
and row 0 zeroed (its true contribution underflows to 0 in f32 anyway).
E and the p state are kept in bf16 so the per-step matmul is a single
1-pass PE instruction (fp32 moving operands cost 2 half-speed passes and
double-width weight loads); the log-domain outputs only see the bf16
quantization as ~2^-9 relative noise on p, a ~1e-5 absolute error.
Per step this is one PE matmul (E stationary) + one DVE multiply; the ln,
the K*t correction, output transposes and DMAs are bulk work off the
sequential chain.
"""

import numpy as np

N = 64
T = 512
B = 32
NCORES = 8
BS = B // NCORES  # 4 batch elements per core
K = 4.66


def _build_program():
    import concourse.bass as bass
    import concourse.mybir as mybir

    FT = mybir.dt.float32
    BF = mybir.dt.bfloat16
    AF = mybir.ActivationFunctionType

    nc = bass.Bass()
    sc_d = nc.declare_dram_parameter("sc", [BS * T, N], FT, isOutput=False)
    tr_d = nc.declare_dram_parameter("tr", [N, N], FT, isOutput=False)
    trt_d = nc.declare_dram_parameter("trt", [N, N], FT, isOutput=False)
    ktc_d = nc.declare_dram_parameter("ktc", [N, T], FT, isOutput=False)
    id_d = nc.declare_dram_parameter("ident", [128, 128], FT, isOutput=False)
    kc_d = nc.declare_dram_parameter("kconst", [N, 2], FT, isOutput=False)
    out_d = nc.declare_dram_parameter("out", [BS * T, N], FT, isOutput=True)

    from contextlib import ExitStack

    with ExitStack() as ctx:
        sc_nat = ctx.enter_context(nc.sbuf_tensor([128, 16 * N], FT))
        es_all = ctx.enter_context(nc.sbuf_tensor([N, T * BS], FT))
        p_all = ctx.enter_context(nc.sbuf_tensor([N, T * BS], BF))
        out_sb = ctx.enter_context(nc.sbuf_tensor([N, T * BS], FT))
        e_sb = ctx.enter_context(nc.sbuf_tensor([N, N], BF))
        tr_nat = ctx.enter_context(nc.sbuf_tensor([N, N], FT))
        tr_t = ctx.enter_context(nc.sbuf_tensor([N, N], FT))
        e0k = ctx.enter_context(nc.sbuf_tensor([N, 1], FT))
        ktc_sb = ctx.enter_context(nc.sbuf_tensor([N, T], FT))
        ident = ctx.enter_context(nc.sbuf_tensor([128, 128], FT))
        out_tr = ctx.enter_context(nc.sbuf_tensor([128, 16 * N], FT))
        kc_sb = ctx.enter_context(nc.sbuf_tensor([N, 2], FT))
        tp0 = ctx.enter_context(nc.psum_tensor([N, 128], FT))
        tp1 = ctx.enter_context(nc.psum_tensor([N, 128], FT))
        s_ps = ctx.enter_context(nc.psum_tensor([N, BS], FT))
        tq0 = ctx.enter_context(nc.psum_tensor([128, N], FT))
        tq1 = ctx.enter_context(nc.psum_tensor([128, N], FT))
        dma_sem = ctx.enter_context(nc.semaphore())
        acte_sem = ctx.enter_context(nc.semaphore())
        act_sem = ctx.enter_context(nc.semaphore())
        dve_sem = ctx.enter_context(nc.semaphore())
        pe_sem = ctx.enter_context(nc.semaphore())
        actln_sem = ctx.enter_context(nc.semaphore())
        dvek_sem = ctx.enter_context(nc.semaphore())
        pe2_sem = ctx.enter_context(nc.semaphore())
        out_sem = ctx.enter_context(nc.semaphore())
        actcp_sem = ctx.enter_context(nc.semaphore())
        block = ctx.enter_context(nc.Block())
        tp = [tp0, tp1]
        tq = [tq0, tq1]
        # t-major free layout [j, t*BS + b]: per-step slices are contiguous,
        # per-(b, t-chunk) views are stride-BS
        esw = es_all[:, :].rearrange("p (t b) -> p b t", b=BS)
        ow = out_sb[:, :].rearrange("p (t b) -> p b t", b=BS)

        @block.sync
        def _(sync):
            sync.dma_start(
                sc_nat[:, :].rearrange("p (k j) -> p k j", j=N),
                sc_d[:, :].rearrange("(k p) j -> p k j", p=128),
            ).then_inc(dma_sem, 16)
            sync.dma_start(tr_nat[:, :], tr_d[:, :]).then_inc(dma_sem, 16)
            sync.dma_start(tr_t[:, :], trt_d[:, :]).then_inc(dma_sem, 16)
            sync.dma_start(ktc_sb[:, :], ktc_d[:, :]).then_inc(dma_sem, 16)
            sync.dma_start(ident[:, :], id_d[:, :]).then_inc(dma_sem, 16)
            sync.dma_start(kc_sb[:, :], kc_d[:, :]).then_inc(dma_sem, 16)
            out_v = out_d[:, :].rearrange("(k p) j -> k p j", p=128)
            for k in range(16):
                sync.wait_ge(actcp_sem, k + 1)
                sync.dma_start(
                    out_v[k], out_tr[:, k * N : (k + 1) * N]
                ).then_inc(out_sem, 16)

        @block.tensor
        def _(tensor):
            tensor.wait_ge(dma_sem, 96)
            # scores tiles [128(bt), 64(j)] -> psum [64(j), 128(t-sub)]
            for k in range(16):
                if k >= 2:
                    tensor.wait_ge(act_sem, k - 1)
                tensor.transpose(
                    tp[k % 2][:, :], sc_nat[:, k * N : (k + 1) * N], ident[:, :]
                ).then_inc(pe_sem, 1)
            # sequential scan: s = E^T @ p_{t-1}, E stationary bf16, 1 pass.
            # The wait is attached to the matmul itself (no standalone
            # EventSemaphore instruction on the chain).
            for t in range(1, T):
                mm = tensor.matmul(
                    s_ps[:, :], e_sb[:, :], p_all[:, (t - 1) * BS : t * BS]
                )
                mm._wait_ge(dve_sem, t)
                mm.then_inc(pe_sem, 1)
            # output transposes [64(j), 128(t-sub)] -> psum [128(t-sub), 64(j)]
            for k in range(16):
                b, tc = k // 4, k % 4
                tensor.wait_ge(dvek_sem, b + 1)
                if k >= 2:
                    tensor.wait_ge(actcp_sem, k - 1)
                tensor.transpose(
                    tq[k % 2][:, :], ow[:, b, tc * 128 : (tc + 1) * 128],
                    ident[0:N, 0:N],
                ).then_inc(pe2_sem, 1)

        @block.scalar
        def _(scalar):
            scalar.wait_ge(dma_sem, 96)
            scalar.activation(e_sb[:, :], tr_nat[:, :], AF.Exp).then_inc(acte_sem, 1)
            scalar.activation(
                e0k[:, :], tr_t[:, 0:1], AF.Exp, bias=kc_sb[:, 0:1]
            ).then_inc(acte_sem, 1)
            for k in range(16):
                b, tc = k // 4, k % 4
                scalar.wait_ge(pe_sem, k + 1)
                scalar.activation(
                    esw[:, b, tc * 128 : (tc + 1) * 128], tp[k % 2][:, :], AF.Exp,
                    bias=kc_sb[:, 1:2],
                ).then_inc(act_sem, 1)
            scalar.wait_ge(dve_sem, T)
            scalar.activation(out_sb[:, :], p_all[:, :], AF.Ln).then_inc(
                actln_sem, 1
            )
            for k in range(16):
                scalar.wait_ge(pe2_sem, k + 1)
                scalar.copy(
                    out_tr[:, k * N : (k + 1) * N], tq[k % 2][:, :]
                ).then_inc(actcp_sem, 1)

        @block.vector
        def _(vector):
            vector.wait_ge(acte_sem, 2)
            vector.wait_ge(act_sem, 16)
            vector.memset(e_sb[:, 0:1], 1.0)
            vector.memset(e_sb[0:1, :], 0.0)
            vector.memset(e0k[0:1, 0:1], float(np.exp(K)))
            vector.tensor_scalar_mul(
                p_all[:, 0:BS], es_all[:, 0:BS], e0k[:, :]
            ).then_inc(dve_sem, 1)
            for t in range(1, T):
                mul = vector.tensor_mul(
                    p_all[:, t * BS : (t + 1) * BS],
                    s_ps[:, :],
                    es_all[:, t * BS : (t + 1) * BS],
                )
                mul._wait_ge(pe_sem, 16 + t)
                mul.then_inc(dve_sem, 1)
            for c in range(BS):
                vector.wait_ge(actln_sem, 1)
                vector.tensor_add(
                    ow[:, c, :], ow[:, c, :], ktc_sb[:, :]
                ).then_inc(dvek_sem, 1)

    return nc


LAST_RESULT = None


def kernel(scores: np.ndarray, transitions: np.ndarray) -> np.ndarray:
    global LAST_RESULT
    from concourse.bass_utils import run_bass_kernel_spmd

    scores = np.ascontiguousarray(scores, dtype=np.float32)
    transitions = np.ascontiguousarray(transitions, dtype=np.float32)

    ktc = (K * np.arange(T, dtype=np.float32))[None, :] * np.ones(
        (N, 1), dtype=np.float32
    )
    ktc[0, :] -= 10000.0
    ident = np.eye(128, dtype=np.float32)
    kconst = np.stack([np.full(N, K, np.float32), np.full(N, -K, np.float32)], axis=1)
    trt = np.ascontiguousarray(transitions.T)

    nc = _build_program()
    in_maps = []
    for c in range(NCORES):
        shard = np.ascontiguousarray(
            scores[c * BS : (c + 1) * BS].reshape(BS * T, N)
        )
        in_maps.append(
            {"sc": shard, "tr": transitions, "trt": trt, "ktc": ktc,
             "ident": ident, "kconst": kconst}
        )
    res = run_bass_kernel_spmd(nc, in_maps, list(range(NCORES)))
    LAST_RESULT = res
    out = np.empty((B, T, N), dtype=np.float32)
    for c in range(NCORES):
        out[c * BS : (c + 1) * BS] = res.results[c]["out"].reshape(BS, T, N)
    return out
